# revision 15
# baseline (speedup 1.0000x reference)
"""Node2Node supervised-contrastive loss on 8 Trainium2 NeuronCores.

Hybrid per-core pipeline (data-parallel over the x table):
  - Core c owns x rows [c*SL, (c+1)*SL), reordered host-side by class label.
    It normalizes its slice on-device into (a) a row-layout bf16 DRAM table
    for gathers and (b) a TensorE-transposed SBUF copy tableT [D, SLP].
  - Anchors are slot-permuted per core by (class, per-core sample count).
    Blocks of 128 slots that are class-PURE are processed with a dense
    cross product: matmul afT_block [D,128] x tableT [D,512-row chunks]
    -> sims of all 128 anchors vs every slice row, exp on ScalarE, then one
    fused scalar_tensor_tensor per chunk against a host-built bf16 weight
    mask (sample multiplicities; zero elsewhere) accumulates the
    denominator.  The numerator needs no second mask: rows are class-sorted,
    so it is the same masked sum restricted to the block's class row-range.
  - The remaining MIXED blocks (class boundaries) use the gather path:
    transposed dma_gather (DRAM source) pulls each pair's feature column,
    vector multiplies by afT, TensorE reduces each 128-pair column with a
    ones-rhs matmul into PSUM (partition = anchor slot), ScalarE exps into
    an E matrix, and per-block scalar_tensor_tensor with host masks gives
    numerator/denominator.
  - A second tiny launch combines the 8 cores' partials with host counts:
    -log(num/den)/cnt, summed on-device.
"""

import os
import sys

import numpy as np
import ml_dtypes

sys.path.insert(0, "/opt/trn_rl_repo")

import concourse.bass as bass
import concourse.bacc as bacc
import concourse.mybir as mybir
import concourse.tile as tile
from concourse import bass_utils

F32 = mybir.dt.float32
BF16 = mybir.dt.bfloat16
I16 = mybir.dt.int16
MUL = mybir.AluOpType.mult
ADD = mybir.AluOpType.add
SUB = mybir.AluOpType.subtract
EQ = mybir.AluOpType.is_equal
AFT = mybir.ActivationFunctionType


class CFG:
    def __init__(self, N=100000, D=128, A=4096, S=512, NC=8, TEMP=0.1):
        self.N, self.D, self.A, self.S, self.NC, self.TEMP = N, D, A, S, NC, TEMP
        self.SL = N // NC
        self.NB = A // 128
        self.CALL = 36                 # gather-path columns per dma_gather
        self.RC = 512                  # cross-path rows per matmul chunk


REAL = CFG()


# --------------------------------------------------------------------------
# host-side prep (integer bookkeeping only)
# --------------------------------------------------------------------------

def prep(cfg, x, y, anchors, sampled):
    N, A, S, NC, SL, NB = cfg.N, cfg.A, cfg.S, cfg.NC, cfg.SL, cfg.NB
    x = np.ascontiguousarray(np.asarray(x, dtype=np.float32))
    y64 = np.asarray(y, dtype=np.int64)
    anchors = np.asarray(anchors, dtype=np.int64)
    sampled = np.asarray(sampled, dtype=np.int64)

    y_a = y64[anchors]                                  # [A]
    posm = (y64[sampled] == y_a[:, None])               # [A, S]
    cnt_pos = posm.sum(1).astype(np.float32)
    cntg = cnt_pos.reshape(NB, 128).T.copy()            # [128, NB] orig order

    core_of = sampled // SL
    cnt = np.zeros((A, NC), dtype=np.int64)
    for c in range(NC):
        cnt[:, c] = (core_of == c).sum(1)

    # per-core slot permutation: class-major, count-minor (stable)
    perms, ranks = [], []
    Ms = np.zeros((NC, NB), dtype=np.int64)
    for c in range(NC):
        p = np.lexsort((cnt[:, c], y_a))
        r = np.empty(A, dtype=np.int64)
        r[p] = np.arange(A)
        perms.append(p)
        ranks.append(r)
        Ms[c] = cnt[p, c].reshape(NB, 128).max(1)
    M_all = Ms.max(0)

    # block class map (same for all cores: class-major sort, same class sizes)
    blk_cls = y_a[perms[0]].reshape(NB, 128)
    pure = [int(np.unique(blk_cls[j]).size) == 1 for j in range(NB)]
    cross_blocks = [j for j in range(NB) if pure[j]]
    gath_blocks = [j for j in range(NB) if not pure[j]]

    # class-grid row layout: class c occupies the SAME row range on every
    # core (max size over cores, padded); pad rows have zero mask weight
    sizes = np.zeros((NC, 10), dtype=np.int64)
    for c in range(NC):
        yc = y64[c * SL:(c + 1) * SL]
        for k in range(10):
            sizes[c, k] = int((yc == k).sum())
    maxsz = sizes.max(0)
    LO = np.zeros(11, dtype=np.int64)
    LO[1:] = np.cumsum(maxsz)
    SLP = int(-(-LO[10] // 512) * 512)                 # mult of 512 (and 128)
    G = SLP // 128
    cls_rng = [(int(LO[k]), int(LO[k] + maxsz[k])) for k in range(10)]

    row_orders, row_ranks = [], []
    for c in range(NC):
        yc = y64[c * SL:(c + 1) * SL]
        order = np.argsort(yc, kind="stable")           # class-major rows
        # target row of local source row i: LO[class] + pos-within-class
        pos = np.empty(SL, dtype=np.int64)
        tgt = np.empty(SL, dtype=np.int64)
        ofs = 0
        for k in range(10):
            nk = sizes[c, k]
            tgt[order[ofs:ofs + nk]] = LO[k] + np.arange(nk)
            ofs += nk
        row_ranks.append(tgt)
        row_orders.append(order)

    # gather-path geometry: only mixed blocks, packed in their j order
    M = [int(M_all[j]) for j in gath_blocks]
    Cj = np.concatenate([[0], np.cumsum(M)]).astype(int)
    MTOT = int(Cj[-1])
    MTOTP = -(-max(MTOT, 1) // cfg.CALL) * cfg.CALL

    banks = []
    lo_ = 0
    cur = 0
    for m in M:
        if cur + m > 512:
            banks.append((lo_, lo_ + cur))
            lo_ += cur
            cur = 0
        cur += m
    pad = MTOTP - MTOT
    if cur + pad <= 512:
        banks.append((lo_, lo_ + cur + pad))
    else:
        banks.append((lo_, lo_ + cur))
        banks.append((lo_ + cur, lo_ + cur + pad))

    cores = []
    for c in range(NC):
        perm, rank = perms[c], ranks[c]
        rr = row_ranks[c]
        # ---- gather-path pair routing (mixed blocks only) ----
        a_list, s_list = np.nonzero(core_of == c)
        local = rr[sampled[a_list, s_list] - c * SL]
        r = rank[a_list]
        j_of = r // 128
        gsel = ~np.asarray(pure)[j_of]
        ag, sg, lg, rg = a_list[gsel], s_list[gsel], local[gsel], r[gsel]
        order = np.lexsort((lg, ag))
        ag, sg, lg, rg = ag[order], sg[order], lg[order], rg[order]
        # within-anchor position
        n_of = np.zeros(A, dtype=np.int64)
        np.add.at(n_of, ag, 1)
        start = np.concatenate([[0], np.cumsum(n_of)])
        k = np.arange(len(ag)) - start[ag]
        jj = rg // 128
        jloc = np.searchsorted(gath_blocks, jj)         # packed block index
        p = rg % 128
        col = Cj[jloc] + k
        idxmat = np.zeros((128, MTOTP), dtype=np.int16)
        vmask = np.zeros((128, 2, MTOTP), dtype=ml_dtypes.bfloat16)
        idxmat[p, col] = lg.astype(np.int16)
        vmask[p, 0, col] = 1.0
        vmask[p, 1, col] = posm[ag, sg].astype(np.float32)
        flat = idxmat.T.reshape(-1)
        L = flat.size // 16
        wrapped = np.zeros((128, L), dtype=np.int16)
        w16 = flat.reshape(L, 16).T
        for g in range(8):
            wrapped[g * 16:(g + 1) * 16, :] = w16

        # ---- cross-path weight masks (pure blocks) ----
        ac, sc = a_list[~gsel], s_list[~gsel]
        lc = rr[sampled[ac, sc] - c * SL]
        rc = rank[ac]
        wm = np.zeros((len(cross_blocks), 128, SLP), dtype=np.float32)
        jc = np.searchsorted(cross_blocks, rc // 128)
        np.add.at(wm, (jc, rc % 128, lc), 1.0)
        wm = wm.astype(ml_dtypes.bfloat16)

        aperm = anchors[perm]
        xa = x[aperm].reshape(NB, 128, cfg.D).transpose(1, 0, 2)
        xa = np.ascontiguousarray(xa).astype(ml_dtypes.bfloat16)

        xs = np.ones((SLP, cfg.D), dtype=ml_dtypes.bfloat16)
        xs[row_ranks[c]] = x[c * SL:(c + 1) * SL].astype(ml_dtypes.bfloat16)

        cores.append(dict(xs=xs, xa=xa, sidx=wrapped, vmask=vmask, wm=wm))

    nrng = [cls_rng[int(blk_cls[j][0])] for j in cross_blocks]

    meta = dict(M=M, banks=banks, MTOTP=MTOTP, SLP=SLP, G=G,
                cross_blocks=cross_blocks, gath_blocks=gath_blocks,
                nrng=nrng)
    return cores, perms, meta, cntg


# --------------------------------------------------------------------------
# kernel 1
# --------------------------------------------------------------------------

def build_k1(cfg, meta):
    NB, D, CALL, RC = cfg.NB, cfg.D, cfg.CALL, cfg.RC
    SLP, G = meta["SLP"], meta["G"]
    M, banks, MTOTP = meta["M"], meta["banks"], meta["MTOTP"]
    cross_blocks = meta["cross_blocks"]
    gath_blocks = meta["gath_blocks"]
    KC = len(cross_blocks)
    Cj = np.concatenate([[0], np.cumsum(M)]).astype(int)
    MTOT = int(Cj[-1])
    NCALL = MTOTP // CALL
    NRC = SLP // RC                    # cross row chunks (512 each)

    bank_of = np.zeros(MTOTP, dtype=int)
    for b, (lo, hi) in enumerate(banks):
        bank_of[lo:hi] = b
    blk_of = np.zeros(MTOTP, dtype=int)
    for i, j in enumerate(gath_blocks):
        blk_of[Cj[i]:Cj[i + 1]] = j
    if MTOT < MTOTP:
        blk_of[MTOT:] = gath_blocks[-1] if gath_blocks else 0

    nrng = meta["nrng"]

    nc = bacc.Bacc("TRN2", target_bir_lowering=False, debug=False,
                   num_devices=cfg.NC, num_swdge_queues=4)
    xs = nc.dram_tensor("xs", [SLP, D], BF16, kind="ExternalInput").ap()
    xa = nc.dram_tensor("xa", [128, NB, D], BF16, kind="ExternalInput").ap()
    sidx = nc.dram_tensor("sidx", [128, MTOTP * 8], I16,
                          kind="ExternalInput").ap()
    vmask = nc.dram_tensor("vmask", [128, 2, MTOTP], BF16,
                           kind="ExternalInput").ap()
    wm = nc.dram_tensor("wm", [KC, 128, SLP], BF16, kind="ExternalInput").ap()
    ident = nc.dram_tensor("ident", [128, 128], BF16, kind="ExternalInput").ap()
    acc_out = nc.dram_tensor("acc", [128, NB, 2], F32,
                             kind="ExternalOutput").ap()

    with tile.TileContext(nc) as tc:
        with tc.tile_pool(name="pers", bufs=1) as pers, \
             tc.tile_pool(name="dpool", bufs=1, space="DRAM") as dpool:
            table = dpool.tile([SLP, D], BF16)          # row-layout, DRAM
            tableT = pers.tile([128, SLP], BF16)        # transposed, SBUF
            sidxT = pers.tile([128, MTOTP * 8], I16)
            nc.sync.dma_start(sidxT[:], sidx[:])
            vmT = pers.tile([128, 2, MTOTP], BF16)
            nc.sync.dma_start(vmT[:], vmask[:])
            idT = pers.tile([128, 128], BF16)
            nc.sync.dma_start(idT[:], ident[:])
            ones = pers.tile([128, 1], BF16)
            nc.vector.memset(ones[:], 1.0)
            afT = pers.tile([128, NB, D], BF16)
            E = pers.tile([128, MTOTP], BF16)
            accT = pers.tile([128, NB, 2], F32)
            junk = pers.tile([128, 2048], BF16)

            # ---- phase A: normalize slice; write DRAM table + SBUF tableT
            xsr = xs.rearrange("(g p) d -> p g d", p=128)
            tbr = table[:].rearrange("(g p) d -> p g d", p=128)
            with tc.tile_pool(name="pa", bufs=1) as pa, \
                 tc.tile_pool(name="pap", bufs=1, space="PSUM") as pap:
                CH = 13
                for h in range(-(-G // CH)):
                    g0 = h * CH
                    ch = min(CH, G - g0)
                    xt = pa.tile([128, CH, D], BF16, tag=f"xt{h % 2}")
                    nc.sync.dma_start(xt[:, :ch, :], xsr[:, g0:g0 + ch, :])
                    sq = pa.tile([128, CH, D], F32, tag=f"sq{h % 2}")
                    nc.scalar.activation(sq[:, :ch, :], xt[:, :ch, :],
                                         AFT.Square)
                    nrm = pa.tile([128, CH], F32, tag=f"n{h % 2}")
                    nc.vector.reduce_sum(out=nrm[:, :ch], in_=sq[:, :ch, :],
                                         axis=mybir.AxisListType.X)
                    sr = pa.tile([128, CH], F32, tag=f"s{h % 2}")
                    nc.scalar.activation(sr[:, :ch], nrm[:, :ch], AFT.Sqrt)
                    rs = pa.tile([128, CH], F32, tag=f"r{h % 2}")
                    nc.vector.reciprocal(rs[:, :ch], sr[:, :ch])
                    xn = pa.tile([128, CH, D], BF16, tag=f"x{h % 2}")
                    nc.vector.tensor_tensor(
                        out=xn[:, :ch, :], in0=xt[:, :ch, :],
                        in1=rs[:, :ch].unsqueeze(2).to_broadcast([128, ch, D]),
                        op=MUL)
                    nc.sync.dma_start(tbr[:, g0:g0 + ch, :], xn[:, :ch, :])
                    for g in range(ch):
                        pt = pap.tile([128, 128], BF16, tag=f"p{g % 4}")
                        nc.tensor.transpose(pt[:], xn[:, g, :], idT[:])
                        col = (g0 + g) * 128
                        if g % 2 == 0:
                            nc.scalar.activation(tableT[:, col:col + 128],
                                                 pt[:], AFT.Copy)
                        else:
                            nc.vector.tensor_copy(
                                out=tableT[:, col:col + 128], in_=pt[:])

                # anchors: normalize + transpose into afT
                xat = pa.tile([128, NB, D], BF16)
                nc.sync.dma_start(xat[:], xa[:])
                sqa = pa.tile([128, NB, D], F32)
                nc.scalar.activation(sqa[:], xat[:], AFT.Square)
                nrma = pa.tile([128, NB], F32)
                nc.vector.reduce_sum(out=nrma[:], in_=sqa[:],
                                     axis=mybir.AxisListType.X)
                sra = pa.tile([128, NB], F32)
                nc.scalar.activation(sra[:], nrma[:], AFT.Sqrt)
                rsa = pa.tile([128, NB], F32)
                nc.vector.reciprocal(rsa[:], sra[:])
                afn = pa.tile([128, NB, D], BF16)
                nc.vector.tensor_tensor(
                    out=afn[:], in0=xat[:],
                    in1=rsa[:].unsqueeze(2).to_broadcast([128, NB, D]), op=MUL)
                for j in range(NB):
                    pt = pap.tile([128, 128], BF16, tag=f"p{j % 4}")
                    nc.tensor.transpose(pt[:], afn[:, j, :], idT[:])
                    nc.vector.tensor_copy(out=afT[:, j, :], in_=pt[:])

            # ---- phase X: cross blocks (dense, no gather) ----
            # 2048-row rounds; whole-block mask preloaded in one DMA
            BR = 4 * RC
            NBR = -(-SLP // BR)
            with tc.tile_pool(name="px", bufs=1) as px, \
                 tc.tile_pool(name="pxp", bufs=1, space="PSUM") as pxp:
                for kj, j in enumerate(cross_blocks):
                    wmb = px.tile([128, SLP], BF16, tag=f"wm{kj % 2}")
                    nc.sync.dma_start(wmb[:], wm[kj, :, :])
                    acc2 = px.tile([128, 2, NBR], F32, tag=f"a{kj % 2}")
                    nc.vector.memset(acc2[:], 0.0)
                    nlo, nhi = nrng[kj]
                    for rch in range(NBR):
                        r0 = rch * BR
                        br = min(BR, SLP - r0)
                        bankC = pxp.tile([128, BR], F32, tag=f"c{rch % 2}")
                        for q in range(br // RC):
                            nc.tensor.matmul(
                                out=bankC[:, q * RC:(q + 1) * RC],
                                lhsT=afT[:, j, :],
                                rhs=tableT[:, r0 + q * RC:r0 + (q + 1) * RC],
                                start=True, stop=True)
                        ec = px.tile([128, BR], BF16, tag=f"e{rch % 2}")
                        nc.scalar.activation(ec[:, :br], bankC[:, :br],
                                             AFT.Exp, scale=1.0 / cfg.TEMP)
                        nc.vector.scalar_tensor_tensor(
                            out=junk[:, :br], in0=ec[:, :br], scalar=1.0,
                            in1=wmb[:, r0:r0 + br], op0=MUL, op1=MUL,
                            accum_out=acc2[:, 1, rch:rch + 1])
                        lo = max(nlo, r0)
                        hi = min(nhi, r0 + br)
                        if lo < hi:
                            nc.vector.scalar_tensor_tensor(
                                out=junk[:, :hi - lo],
                                in0=ec[:, lo - r0:hi - r0], scalar=1.0,
                                in1=wmb[:, lo:hi], op0=MUL, op1=MUL,
                                accum_out=acc2[:, 0, rch:rch + 1])
                    nc.vector.reduce_sum(out=accT[:, j, :], in_=acc2[:],
                                         axis=mybir.AxisListType.X)

            # ---- phase C: gather blocks ----
            if gath_blocks:
                with tc.tile_pool(name="pc", bufs=1) as pc, \
                     tc.tile_pool(name="pcp", bufs=1, space="PSUM") as pcp:
                    bank_tiles = {}
                    for i in range(NCALL):
                        c0 = i * CALL
                        gt = pc.tile([128, 1, CALL * D], BF16, tag=f"g{i % 4}")
                        nc.gpsimd.dma_gather(
                            gt[:], table[:],
                            sidxT[:, c0 * 8:(c0 + CALL) * 8],
                            CALL * 128, CALL * 128, D, transpose=True,
                            single_packet=False, queue_num=i % 4)
                        g = gt[:].rearrange("p o (c q) -> p (o c) q", q=128)
                        prod = pc.tile([128, CALL, D], BF16, tag=f"pr{i % 2}")
                        s = 0
                        while s < CALL:
                            j = blk_of[c0 + s]
                            e = s + 1
                            while e < CALL and blk_of[c0 + e] == j:
                                e += 1
                            nc.vector.tensor_tensor(
                                out=prod[:, s:e, :], in0=g[:, s:e, :],
                                in1=afT[:, j, :].unsqueeze(1).to_broadcast(
                                    [128, e - s, D]),
                                op=MUL)
                            s = e
                        for s in range(CALL):
                            fc = c0 + s
                            b = bank_of[fc]
                            lo, hi = banks[b]
                            if b not in bank_tiles:
                                bt = pcp.tile([128, 512], F32, tag=f"b{b % 4}")
                                bank_tiles[b] = bt
                            bt = bank_tiles[b]
                            nc.tensor.matmul(out=bt[:, fc - lo:fc - lo + 1],
                                             lhsT=prod[:, s, :], rhs=ones[:],
                                             start=True, stop=True)
                            if fc == hi - 1:
                                nc.scalar.activation(
                                    E[:, lo:hi], bt[:, :hi - lo],
                                    AFT.Exp, scale=1.0 / cfg.TEMP)
                                del bank_tiles[b]

                    for i, j in enumerate(gath_blocks):
                        lo, hi = int(Cj[i]), int(Cj[i + 1])
                        w = hi - lo
                        nc.vector.scalar_tensor_tensor(
                            out=junk[:, :w], in0=E[:, lo:hi], scalar=1.0,
                            in1=vmT[:, 1, lo:hi], op0=MUL, op1=MUL,
                            accum_out=accT[:, j, 0:1])
                        nc.vector.scalar_tensor_tensor(
                            out=junk[:, :w], in0=E[:, lo:hi], scalar=1.0,
                            in1=vmT[:, 0, lo:hi], op0=MUL, op1=MUL,
                            accum_out=accT[:, j, 1:2])
            nc.sync.dma_start(acc_out[:], accT[:])
    nc.compile()
    return nc


# --------------------------------------------------------------------------
# kernel 2
# --------------------------------------------------------------------------

def build_k2(cfg):
    NB, NC = cfg.NB, cfg.NC
    nc = bacc.Bacc("TRN2", target_bir_lowering=False, debug=False, num_devices=1)
    parts = nc.dram_tensor("parts", [128, NC, NB, 2], F32,
                           kind="ExternalInput").ap()
    cnt = nc.dram_tensor("cnt", [128, NB], F32, kind="ExternalInput").ap()
    out = nc.dram_tensor("out", [1, 1], F32, kind="ExternalOutput").ap()
    with tile.TileContext(nc) as tc:
        with tc.tile_pool(name="p", bufs=1) as p, \
             tc.tile_pool(name="ps", bufs=1, space="PSUM") as psp:
            t = p.tile([128, NC, NB, 2], F32)
            nc.sync.dma_start(t[:], parts[:])
            ct = p.tile([128, NB], F32)
            nc.sync.dma_start(ct[:], cnt[:])
            s3 = p.tile([128, NB, 2], F32)
            tt = t[:].transpose([0, 2, 3, 1])
            nc.vector.reduce_sum(out=s3[:], in_=tt, axis=mybir.AxisListType.X)
            n_ = s3[:, :, 0]
            d_ = s3[:, :, 1]
            cz = p.tile([128, NB], F32)
            nc.vector.tensor_scalar(out=cz[:], in0=ct[:], scalar1=0.0,
                                    scalar2=None, op0=EQ)
            n1 = p.tile([128, NB], F32)
            nc.vector.tensor_tensor(out=n1[:], in0=n_, in1=cz[:], op=ADD)
            c1 = p.tile([128, NB], F32)
            nc.vector.tensor_scalar_max(out=c1[:], in0=ct[:], scalar1=1.0)
            lnn = p.tile([128, NB], F32)
            nc.scalar.activation(lnn[:], n1[:], AFT.Ln)
            lnd = p.tile([128, NB], F32)
            nc.scalar.activation(lnd[:], d_, AFT.Ln)
            df = p.tile([128, NB], F32)
            nc.vector.tensor_tensor(out=df[:], in0=lnd[:], in1=lnn[:], op=SUB)
            rc = p.tile([128, NB], F32)
            nc.vector.reciprocal(rc[:], c1[:])
            pa = p.tile([128, NB], F32)
            nc.vector.tensor_tensor(out=pa[:], in0=df[:], in1=rc[:], op=MUL)
            m = p.tile([128, NB], F32)
            nc.scalar.activation(m[:], cz[:], AFT.Copy, scale=-1.0, bias=1.0)
            pa2 = p.tile([128, NB], F32)
            nc.vector.tensor_tensor(out=pa2[:], in0=pa[:], in1=m[:], op=MUL)
            rs = p.tile([128, 1], F32)
            nc.vector.reduce_sum(out=rs[:], in_=pa2[:], axis=mybir.AxisListType.X)
            one = p.tile([128, 1], F32)
            nc.vector.memset(one[:], 1.0)
            acc = psp.tile([1, 1], F32)
            nc.tensor.matmul(out=acc[:], lhsT=rs[:], rhs=one[:], start=True,
                             stop=True)
            res = p.tile([1, 1], F32)
            nc.vector.tensor_copy(out=res[:], in_=acc[:])
            nc.sync.dma_start(out[:], res[:])
    nc.compile()
    return nc


# --------------------------------------------------------------------------
# entry point
# --------------------------------------------------------------------------

def _run(cfg, x, y, anchors, sampled, trace=False, sim=False):
    cores, perms, meta, cntg = prep(cfg, x, y, anchors, sampled)
    nc1 = build_k1(cfg, meta)
    eye = np.eye(128, dtype=ml_dtypes.bfloat16)
    in_maps = [dict(xs=c["xs"], xa=c["xa"], sidx=c["sidx"], vmask=c["vmask"],
                    wm=c["wm"], ident=eye) for c in cores]
    if sim:
        from concourse.bass_interp import CoreSim
        s = CoreSim(nc1, require_finite=False, require_nnan=False)
        for k, v in in_maps[0].items():
            s.tensor(k)[:] = v
        s.simulate(check_with_hw=False)
        results = [{"acc": np.array(s.tensor("acc"))}]
        r1 = None
    else:
        kw = dict(trace=True, trace_cores=list(range(cfg.NC)),
                  stitch_traces=False) if trace else {}
        r1 = bass_utils.run_bass_kernel_spmd(nc1, in_maps,
                                             core_ids=list(range(cfg.NC)), **kw)
        results = r1.results
    aligned = np.zeros((cfg.NC, cfg.A, 2), dtype=np.float32)
    for c in range(len(results)):
        acc = results[c]["acc"]
        acc_t = acc.transpose(1, 0, 2).reshape(cfg.A, 2)
        aligned[c, perms[c]] = acc_t
    parts = aligned.reshape(cfg.NC, cfg.NB, 128, 2).transpose(2, 0, 1, 3).copy()
    if sim:
        return None, None, aligned
    nc2 = build_k2(cfg)
    r2 = bass_utils.run_bass_kernel_spmd(nc2, [dict(parts=parts, cnt=cntg)],
                                         core_ids=[0])
    val = np.float32(r2.results[0]["out"].reshape(-1)[0])
    return val, r1, aligned


def kernel(x, y, anchors, sampled):
    val, _, _ = _run(REAL, np.asarray(x), np.asarray(y), np.asarray(anchors),
                     np.asarray(sampled),
                     trace=os.environ.get("K_TRACE", "0") == "1")
    return np.asarray(val, dtype=np.float32)


# revision 16
# speedup vs baseline: 1.0264x; 1.0264x over previous
"""Node2Node supervised-contrastive loss on 8 Trainium2 NeuronCores.

Hybrid per-core pipeline (data-parallel over the x table):
  - Core c owns x rows [c*SL, (c+1)*SL), reordered host-side by class label.
    It normalizes its slice on-device into (a) a row-layout bf16 DRAM table
    for gathers and (b) a TensorE-transposed SBUF copy tableT [D, SLP].
  - Anchors are slot-permuted per core by (class, per-core sample count).
    Blocks of 128 slots that are class-PURE are processed with a dense
    cross product: matmul afT_block [D,128] x tableT [D,512-row chunks]
    -> sims of all 128 anchors vs every slice row, exp on ScalarE, then one
    fused scalar_tensor_tensor per chunk against a host-built bf16 weight
    mask (sample multiplicities; zero elsewhere) accumulates the
    denominator.  The numerator needs no second mask: rows are class-sorted,
    so it is the same masked sum restricted to the block's class row-range.
  - The remaining MIXED blocks (class boundaries) use the gather path:
    transposed dma_gather (DRAM source) pulls each pair's feature column,
    vector multiplies by afT, TensorE reduces each 128-pair column with a
    ones-rhs matmul into PSUM (partition = anchor slot), ScalarE exps into
    an E matrix, and per-block scalar_tensor_tensor with host masks gives
    numerator/denominator.
  - A second tiny launch combines the 8 cores' partials with host counts:
    -log(num/den)/cnt, summed on-device.
"""

import os
import sys

import numpy as np
import ml_dtypes

sys.path.insert(0, "/opt/trn_rl_repo")

import concourse.bass as bass
import concourse.bacc as bacc
import concourse.mybir as mybir
import concourse.tile as tile
from concourse import bass_utils

F32 = mybir.dt.float32
BF16 = mybir.dt.bfloat16
I16 = mybir.dt.int16
MUL = mybir.AluOpType.mult
ADD = mybir.AluOpType.add
SUB = mybir.AluOpType.subtract
EQ = mybir.AluOpType.is_equal
AFT = mybir.ActivationFunctionType


class CFG:
    def __init__(self, N=100000, D=128, A=4096, S=512, NC=8, TEMP=0.1):
        self.N, self.D, self.A, self.S, self.NC, self.TEMP = N, D, A, S, NC, TEMP
        self.SL = N // NC
        self.NB = A // 128
        self.CALL = 36                 # gather-path columns per dma_gather
        self.RC = 512                  # cross-path rows per matmul chunk


REAL = CFG()


# --------------------------------------------------------------------------
# host-side prep (integer bookkeeping only)
# --------------------------------------------------------------------------

def prep(cfg, x, y, anchors, sampled):
    N, A, S, NC, SL, NB = cfg.N, cfg.A, cfg.S, cfg.NC, cfg.SL, cfg.NB
    x = np.ascontiguousarray(np.asarray(x, dtype=np.float32))
    y64 = np.asarray(y, dtype=np.int64)
    anchors = np.asarray(anchors, dtype=np.int64)
    sampled = np.asarray(sampled, dtype=np.int64)

    y_a = y64[anchors]                                  # [A]
    posm = (y64[sampled] == y_a[:, None])               # [A, S]
    cnt_pos = posm.sum(1).astype(np.float32)
    cntg = cnt_pos.reshape(NB, 128).T.copy()            # [128, NB] orig order

    core_of = sampled // SL
    cnt = np.zeros((A, NC), dtype=np.int64)
    for c in range(NC):
        cnt[:, c] = (core_of == c).sum(1)

    # per-core slot permutation: class-major, count-minor (stable)
    perms, ranks = [], []
    Ms = np.zeros((NC, NB), dtype=np.int64)
    for c in range(NC):
        p = np.lexsort((cnt[:, c], y_a))
        r = np.empty(A, dtype=np.int64)
        r[p] = np.arange(A)
        perms.append(p)
        ranks.append(r)
        Ms[c] = cnt[p, c].reshape(NB, 128).max(1)
    M_all = Ms.max(0)

    # block class map (same for all cores: class-major sort, same class sizes)
    blk_cls = y_a[perms[0]].reshape(NB, 128)
    pure = [int(np.unique(blk_cls[j]).size) == 1 for j in range(NB)]
    cross_blocks = [j for j in range(NB) if pure[j]]
    gath_blocks = [j for j in range(NB) if not pure[j]]

    # class-grid row layout: class c occupies the SAME row range on every
    # core (max size over cores, padded); pad rows have zero mask weight
    sizes = np.zeros((NC, 10), dtype=np.int64)
    for c in range(NC):
        yc = y64[c * SL:(c + 1) * SL]
        for k in range(10):
            sizes[c, k] = int((yc == k).sum())
    maxsz = sizes.max(0)
    LO = np.zeros(11, dtype=np.int64)
    LO[1:] = np.cumsum(maxsz)
    SLP = int(-(-LO[10] // 512) * 512)                 # mult of 512 (and 128)
    G = SLP // 128
    cls_rng = [(int(LO[k]), int(LO[k] + maxsz[k])) for k in range(10)]

    row_orders, row_ranks = [], []
    for c in range(NC):
        yc = y64[c * SL:(c + 1) * SL]
        order = np.argsort(yc, kind="stable")           # class-major rows
        # target row of local source row i: LO[class] + pos-within-class
        pos = np.empty(SL, dtype=np.int64)
        tgt = np.empty(SL, dtype=np.int64)
        ofs = 0
        for k in range(10):
            nk = sizes[c, k]
            tgt[order[ofs:ofs + nk]] = LO[k] + np.arange(nk)
            ofs += nk
        row_ranks.append(tgt)
        row_orders.append(order)

    # gather-path geometry: only mixed blocks, packed in their j order
    M = [int(M_all[j]) for j in gath_blocks]
    Cj = np.concatenate([[0], np.cumsum(M)]).astype(int)
    MTOT = int(Cj[-1])
    MTOTP = -(-max(MTOT, 1) // cfg.CALL) * cfg.CALL

    banks = []
    lo_ = 0
    cur = 0
    for m in M:
        if cur + m > 512:
            banks.append((lo_, lo_ + cur))
            lo_ += cur
            cur = 0
        cur += m
    pad = MTOTP - MTOT
    if cur + pad <= 512:
        banks.append((lo_, lo_ + cur + pad))
    else:
        banks.append((lo_, lo_ + cur))
        banks.append((lo_ + cur, lo_ + cur + pad))

    cores = []
    for c in range(NC):
        perm, rank = perms[c], ranks[c]
        rr = row_ranks[c]
        # ---- gather-path pair routing (mixed blocks only) ----
        a_list, s_list = np.nonzero(core_of == c)
        local = rr[sampled[a_list, s_list] - c * SL]
        r = rank[a_list]
        j_of = r // 128
        gsel = ~np.asarray(pure)[j_of]
        ag, sg, lg, rg = a_list[gsel], s_list[gsel], local[gsel], r[gsel]
        order = np.lexsort((lg, ag))
        ag, sg, lg, rg = ag[order], sg[order], lg[order], rg[order]
        # within-anchor position
        n_of = np.zeros(A, dtype=np.int64)
        np.add.at(n_of, ag, 1)
        start = np.concatenate([[0], np.cumsum(n_of)])
        k = np.arange(len(ag)) - start[ag]
        jj = rg // 128
        jloc = np.searchsorted(gath_blocks, jj)         # packed block index
        p = rg % 128
        col = Cj[jloc] + k
        idxmat = np.zeros((128, MTOTP), dtype=np.int16)
        vmask = np.zeros((128, 2, MTOTP), dtype=ml_dtypes.bfloat16)
        idxmat[p, col] = lg.astype(np.int16)
        vmask[p, 0, col] = 1.0
        vmask[p, 1, col] = posm[ag, sg].astype(np.float32)
        flat = idxmat.T.reshape(-1)
        L = flat.size // 16
        wrapped = np.zeros((128, L), dtype=np.int16)
        w16 = flat.reshape(L, 16).T
        for g in range(8):
            wrapped[g * 16:(g + 1) * 16, :] = w16

        # ---- cross-path weight masks (pure blocks) ----
        ac, sc = a_list[~gsel], s_list[~gsel]
        lc = rr[sampled[ac, sc] - c * SL]
        rc = rank[ac]
        wm = np.zeros((len(cross_blocks), 128, SLP), dtype=np.float32)
        jc = np.searchsorted(cross_blocks, rc // 128)
        np.add.at(wm, (jc, rc % 128, lc), 1.0)
        wm = wm.astype(ml_dtypes.bfloat16)

        aperm = anchors[perm]
        xa = x[aperm].reshape(NB, 128, cfg.D).transpose(1, 0, 2)
        xa = np.ascontiguousarray(xa).astype(ml_dtypes.bfloat16)

        xs = np.ones((SLP, cfg.D), dtype=ml_dtypes.bfloat16)
        xs[row_ranks[c]] = x[c * SL:(c + 1) * SL].astype(ml_dtypes.bfloat16)

        cores.append(dict(xs=xs, xa=xa, sidx=wrapped, vmask=vmask, wm=wm))

    nrng = [cls_rng[int(blk_cls[j][0])] for j in cross_blocks]

    meta = dict(M=M, banks=banks, MTOTP=MTOTP, SLP=SLP, G=G,
                cross_blocks=cross_blocks, gath_blocks=gath_blocks,
                nrng=nrng)
    return cores, perms, meta, cntg


# --------------------------------------------------------------------------
# kernel 1
# --------------------------------------------------------------------------

def build_k1(cfg, meta):
    NB, D, CALL, RC = cfg.NB, cfg.D, cfg.CALL, cfg.RC
    SLP, G = meta["SLP"], meta["G"]
    M, banks, MTOTP = meta["M"], meta["banks"], meta["MTOTP"]
    cross_blocks = meta["cross_blocks"]
    gath_blocks = meta["gath_blocks"]
    KC = len(cross_blocks)
    Cj = np.concatenate([[0], np.cumsum(M)]).astype(int)
    MTOT = int(Cj[-1])
    NCALL = MTOTP // CALL
    NRC = SLP // RC                    # cross row chunks (512 each)

    bank_of = np.zeros(MTOTP, dtype=int)
    for b, (lo, hi) in enumerate(banks):
        bank_of[lo:hi] = b
    blk_of = np.zeros(MTOTP, dtype=int)
    for i, j in enumerate(gath_blocks):
        blk_of[Cj[i]:Cj[i + 1]] = j
    if MTOT < MTOTP:
        blk_of[MTOT:] = gath_blocks[-1] if gath_blocks else 0

    nrng = meta["nrng"]

    nc = bacc.Bacc("TRN2", target_bir_lowering=False, debug=False,
                   num_devices=cfg.NC, num_swdge_queues=4)
    xs = nc.dram_tensor("xs", [SLP, D], BF16, kind="ExternalInput").ap()
    xa = nc.dram_tensor("xa", [128, NB, D], BF16, kind="ExternalInput").ap()
    sidx = nc.dram_tensor("sidx", [128, MTOTP * 8], I16,
                          kind="ExternalInput").ap()
    vmask = nc.dram_tensor("vmask", [128, 2, MTOTP], BF16,
                           kind="ExternalInput").ap()
    wm = nc.dram_tensor("wm", [KC, 128, SLP], BF16, kind="ExternalInput").ap()
    ident = nc.dram_tensor("ident", [128, 128], BF16, kind="ExternalInput").ap()
    acc_out = nc.dram_tensor("acc", [128, NB, 2], F32,
                             kind="ExternalOutput").ap()

    with tile.TileContext(nc) as tc:
        with tc.tile_pool(name="pers", bufs=1) as pers, \
             tc.tile_pool(name="dpool", bufs=1, space="DRAM") as dpool:
            table = dpool.tile([SLP, D], BF16)          # row-layout, DRAM
            tableT = pers.tile([128, SLP], BF16)        # transposed, SBUF
            sidxT = pers.tile([128, MTOTP * 8], I16)
            nc.sync.dma_start(sidxT[:], sidx[:])
            vmT = pers.tile([128, 2, MTOTP], BF16)
            nc.sync.dma_start(vmT[:], vmask[:])
            idT = pers.tile([128, 128], BF16)
            nc.sync.dma_start(idT[:], ident[:])
            ones = pers.tile([128, 1], BF16)
            nc.vector.memset(ones[:], 1.0)
            afT = pers.tile([128, NB, D], BF16)
            E = pers.tile([128, MTOTP], BF16)
            accT = pers.tile([128, NB, 2], F32)
            junk = pers.tile([128, 2048], BF16)

            # ---- phase A: normalize slice; write DRAM table + SBUF tableT
            xsr = xs.rearrange("(g p) d -> p g d", p=128)
            tbr = table[:].rearrange("(g p) d -> p g d", p=128)
            with tc.tile_pool(name="pa", bufs=1) as pa, \
                 tc.tile_pool(name="pap", bufs=1, space="PSUM") as pap:
                CH = 13
                for h in range(-(-G // CH)):
                    g0 = h * CH
                    ch = min(CH, G - g0)
                    xt = pa.tile([128, CH, D], BF16, tag=f"xt{h % 2}")
                    nc.sync.dma_start(xt[:, :ch, :], xsr[:, g0:g0 + ch, :])
                    sq = pa.tile([128, CH, D], F32, tag=f"sq{h % 2}")
                    nc.scalar.activation(sq[:, :ch, :], xt[:, :ch, :],
                                         AFT.Square)
                    nrm = pa.tile([128, CH], F32, tag=f"n{h % 2}")
                    nc.vector.reduce_sum(out=nrm[:, :ch], in_=sq[:, :ch, :],
                                         axis=mybir.AxisListType.X)
                    sr = pa.tile([128, CH], F32, tag=f"s{h % 2}")
                    nc.scalar.activation(sr[:, :ch], nrm[:, :ch], AFT.Sqrt)
                    rs = pa.tile([128, CH], F32, tag=f"r{h % 2}")
                    nc.vector.reciprocal(rs[:, :ch], sr[:, :ch])
                    xn = pa.tile([128, CH, D], BF16, tag=f"x{h % 2}")
                    nc.vector.tensor_tensor(
                        out=xn[:, :ch, :], in0=xt[:, :ch, :],
                        in1=rs[:, :ch].unsqueeze(2).to_broadcast([128, ch, D]),
                        op=MUL)
                    nc.sync.dma_start(tbr[:, g0:g0 + ch, :], xn[:, :ch, :])
                    for g in range(ch):
                        pt = pap.tile([128, 128], BF16, tag=f"p{g % 4}")
                        nc.tensor.transpose(pt[:], xn[:, g, :], idT[:])
                        col = (g0 + g) * 128
                        if g % 2 == 0:
                            nc.scalar.activation(tableT[:, col:col + 128],
                                                 pt[:], AFT.Copy)
                        else:
                            nc.vector.tensor_copy(
                                out=tableT[:, col:col + 128], in_=pt[:])

                # anchors: normalize + transpose into afT
                xat = pa.tile([128, NB, D], BF16)
                nc.sync.dma_start(xat[:], xa[:])
                sqa = pa.tile([128, NB, D], F32)
                nc.scalar.activation(sqa[:], xat[:], AFT.Square)
                nrma = pa.tile([128, NB], F32)
                nc.vector.reduce_sum(out=nrma[:], in_=sqa[:],
                                     axis=mybir.AxisListType.X)
                sra = pa.tile([128, NB], F32)
                nc.scalar.activation(sra[:], nrma[:], AFT.Sqrt)
                rsa = pa.tile([128, NB], F32)
                nc.vector.reciprocal(rsa[:], sra[:])
                afn = pa.tile([128, NB, D], BF16)
                nc.vector.tensor_tensor(
                    out=afn[:], in0=xat[:],
                    in1=rsa[:].unsqueeze(2).to_broadcast([128, NB, D]), op=MUL)
                for j in range(NB):
                    pt = pap.tile([128, 128], BF16, tag=f"p{j % 4}")
                    nc.tensor.transpose(pt[:], afn[:, j, :], idT[:])
                    nc.vector.tensor_copy(out=afT[:, j, :], in_=pt[:])

            # ---- phases X+C interleaved: cross blocks + gather calls ----
            BR = 4 * RC
            NBR = -(-SLP // BR)
            with tc.tile_pool(name="px", bufs=1) as px, \
                 tc.tile_pool(name="pxp", bufs=1, space="PSUM") as pxp, \
                 tc.tile_pool(name="pc", bufs=1) as pc, \
                 tc.tile_pool(name="pcp", bufs=1, space="PSUM") as pcp:
                bank_tiles = {}

                def emit_call(i):
                    c0 = i * CALL
                    gt = pc.tile([128, 1, CALL * D], BF16, tag=f"g{i % 4}",
                                 name=f"gt{i}")
                    nc.gpsimd.dma_gather(
                        gt[:], table[:],
                        sidxT[:, c0 * 8:(c0 + CALL) * 8],
                        CALL * 128, CALL * 128, D, transpose=True,
                        single_packet=False, queue_num=i % 4)
                    g = gt[:].rearrange("p o (c q) -> p (o c) q", q=128)
                    prod = pc.tile([128, CALL, D], BF16, tag=f"pr{i % 2}",
                                   name=f"prod{i}")
                    s = 0
                    while s < CALL:
                        j = blk_of[c0 + s]
                        e = s + 1
                        while e < CALL and blk_of[c0 + e] == j:
                            e += 1
                        nc.vector.tensor_tensor(
                            out=prod[:, s:e, :], in0=g[:, s:e, :],
                            in1=afT[:, j, :].unsqueeze(1).to_broadcast(
                                [128, e - s, D]),
                            op=MUL)
                        s = e
                    for s in range(CALL):
                        fc = c0 + s
                        b = bank_of[fc]
                        blo, bhi = banks[b]
                        if b not in bank_tiles:
                            bt = pcp.tile([128, 512], F32, tag=f"b{b % 3}",
                                          name=f"bank{b}")
                            bank_tiles[b] = bt
                        bt = bank_tiles[b]
                        nc.tensor.matmul(out=bt[:, fc - blo:fc - blo + 1],
                                         lhsT=prod[:, s, :], rhs=ones[:],
                                         start=True, stop=True)
                        if fc == bhi - 1:
                            nc.scalar.activation(
                                E[:, blo:bhi], bt[:, :bhi - blo],
                                AFT.Exp, scale=1.0 / cfg.TEMP)
                            del bank_tiles[b]

                ci = 0
                for kj, j in enumerate(cross_blocks):
                    wmb = px.tile([128, SLP], BF16, tag=f"wm{kj % 2}",
                                  name=f"wmb{kj}")
                    nc.sync.dma_start(wmb[:], wm[kj, :, :])
                    acc2 = px.tile([128, 2, NBR], F32, tag=f"a{kj % 2}",
                                   name=f"acc2_{kj}")
                    nc.vector.memset(acc2[:], 0.0)
                    nlo, nhi = nrng[kj]
                    for rch in range(NBR):
                        r0 = rch * BR
                        br = min(BR, SLP - r0)
                        bankC = pxp.tile([128, BR], F32, tag="cx",
                                         name=f"bc{kj}_{rch}")
                        for q in range(br // RC):
                            nc.tensor.matmul(
                                out=bankC[:, q * RC:(q + 1) * RC],
                                lhsT=afT[:, j, :],
                                rhs=tableT[:, r0 + q * RC:r0 + (q + 1) * RC],
                                start=True, stop=True)
                        ec = px.tile([128, BR], BF16, tag=f"e{rch % 2}",
                                     name=f"ec{kj}_{rch}")
                        nc.scalar.activation(ec[:, :br], bankC[:, :br],
                                             AFT.Exp, scale=1.0 / cfg.TEMP)
                        nc.vector.scalar_tensor_tensor(
                            out=junk[:, :br], in0=ec[:, :br], scalar=1.0,
                            in1=wmb[:, r0:r0 + br], op0=MUL, op1=MUL,
                            accum_out=acc2[:, 1, rch:rch + 1])
                        lo = max(nlo, r0)
                        hi = min(nhi, r0 + br)
                        if lo < hi:
                            nc.vector.scalar_tensor_tensor(
                                out=junk[:, :hi - lo],
                                in0=ec[:, lo - r0:hi - r0], scalar=1.0,
                                in1=wmb[:, lo:hi], op0=MUL, op1=MUL,
                                accum_out=acc2[:, 0, rch:rch + 1])
                    nc.vector.reduce_sum(out=accT[:, j, :], in_=acc2[:],
                                         axis=mybir.AxisListType.X)
                    while gath_blocks and ci * (len(cross_blocks)) < \
                            NCALL * (kj + 1):
                        emit_call(ci)
                        ci += 1
                while gath_blocks and ci < NCALL:
                    emit_call(ci)
                    ci += 1

                if gath_blocks:
                    for i, j in enumerate(gath_blocks):
                        lo, hi = int(Cj[i]), int(Cj[i + 1])
                        w = hi - lo
                        nc.vector.scalar_tensor_tensor(
                            out=junk[:, :w], in0=E[:, lo:hi], scalar=1.0,
                            in1=vmT[:, 1, lo:hi], op0=MUL, op1=MUL,
                            accum_out=accT[:, j, 0:1])
                        nc.vector.scalar_tensor_tensor(
                            out=junk[:, :w], in0=E[:, lo:hi], scalar=1.0,
                            in1=vmT[:, 0, lo:hi], op0=MUL, op1=MUL,
                            accum_out=accT[:, j, 1:2])
            nc.sync.dma_start(acc_out[:], accT[:])
    nc.compile()
    return nc


# --------------------------------------------------------------------------
# kernel 2
# --------------------------------------------------------------------------

def build_k2(cfg):
    NB, NC = cfg.NB, cfg.NC
    nc = bacc.Bacc("TRN2", target_bir_lowering=False, debug=False, num_devices=1)
    parts = nc.dram_tensor("parts", [128, NC, NB, 2], F32,
                           kind="ExternalInput").ap()
    cnt = nc.dram_tensor("cnt", [128, NB], F32, kind="ExternalInput").ap()
    out = nc.dram_tensor("out", [1, 1], F32, kind="ExternalOutput").ap()
    with tile.TileContext(nc) as tc:
        with tc.tile_pool(name="p", bufs=1) as p, \
             tc.tile_pool(name="ps", bufs=1, space="PSUM") as psp:
            t = p.tile([128, NC, NB, 2], F32)
            nc.sync.dma_start(t[:], parts[:])
            ct = p.tile([128, NB], F32)
            nc.sync.dma_start(ct[:], cnt[:])
            s3 = p.tile([128, NB, 2], F32)
            tt = t[:].transpose([0, 2, 3, 1])
            nc.vector.reduce_sum(out=s3[:], in_=tt, axis=mybir.AxisListType.X)
            n_ = s3[:, :, 0]
            d_ = s3[:, :, 1]
            cz = p.tile([128, NB], F32)
            nc.vector.tensor_scalar(out=cz[:], in0=ct[:], scalar1=0.0,
                                    scalar2=None, op0=EQ)
            n1 = p.tile([128, NB], F32)
            nc.vector.tensor_tensor(out=n1[:], in0=n_, in1=cz[:], op=ADD)
            c1 = p.tile([128, NB], F32)
            nc.vector.tensor_scalar_max(out=c1[:], in0=ct[:], scalar1=1.0)
            lnn = p.tile([128, NB], F32)
            nc.scalar.activation(lnn[:], n1[:], AFT.Ln)
            lnd = p.tile([128, NB], F32)
            nc.scalar.activation(lnd[:], d_, AFT.Ln)
            df = p.tile([128, NB], F32)
            nc.vector.tensor_tensor(out=df[:], in0=lnd[:], in1=lnn[:], op=SUB)
            rc = p.tile([128, NB], F32)
            nc.vector.reciprocal(rc[:], c1[:])
            pa = p.tile([128, NB], F32)
            nc.vector.tensor_tensor(out=pa[:], in0=df[:], in1=rc[:], op=MUL)
            m = p.tile([128, NB], F32)
            nc.scalar.activation(m[:], cz[:], AFT.Copy, scale=-1.0, bias=1.0)
            pa2 = p.tile([128, NB], F32)
            nc.vector.tensor_tensor(out=pa2[:], in0=pa[:], in1=m[:], op=MUL)
            rs = p.tile([128, 1], F32)
            nc.vector.reduce_sum(out=rs[:], in_=pa2[:], axis=mybir.AxisListType.X)
            one = p.tile([128, 1], F32)
            nc.vector.memset(one[:], 1.0)
            acc = psp.tile([1, 1], F32)
            nc.tensor.matmul(out=acc[:], lhsT=rs[:], rhs=one[:], start=True,
                             stop=True)
            res = p.tile([1, 1], F32)
            nc.vector.tensor_copy(out=res[:], in_=acc[:])
            nc.sync.dma_start(out[:], res[:])
    nc.compile()
    return nc


# --------------------------------------------------------------------------
# entry point
# --------------------------------------------------------------------------

def _run(cfg, x, y, anchors, sampled, trace=False, sim=False):
    cores, perms, meta, cntg = prep(cfg, x, y, anchors, sampled)
    nc1 = build_k1(cfg, meta)
    eye = np.eye(128, dtype=ml_dtypes.bfloat16)
    in_maps = [dict(xs=c["xs"], xa=c["xa"], sidx=c["sidx"], vmask=c["vmask"],
                    wm=c["wm"], ident=eye) for c in cores]
    if sim:
        from concourse.bass_interp import CoreSim
        s = CoreSim(nc1, require_finite=False, require_nnan=False)
        for k, v in in_maps[0].items():
            s.tensor(k)[:] = v
        s.simulate(check_with_hw=False)
        results = [{"acc": np.array(s.tensor("acc"))}]
        r1 = None
    else:
        kw = dict(trace=True, trace_cores=list(range(cfg.NC)),
                  stitch_traces=False) if trace else {}
        r1 = bass_utils.run_bass_kernel_spmd(nc1, in_maps,
                                             core_ids=list(range(cfg.NC)), **kw)
        results = r1.results
    aligned = np.zeros((cfg.NC, cfg.A, 2), dtype=np.float32)
    for c in range(len(results)):
        acc = results[c]["acc"]
        acc_t = acc.transpose(1, 0, 2).reshape(cfg.A, 2)
        aligned[c, perms[c]] = acc_t
    parts = aligned.reshape(cfg.NC, cfg.NB, 128, 2).transpose(2, 0, 1, 3).copy()
    if sim:
        return None, None, aligned
    nc2 = build_k2(cfg)
    r2 = bass_utils.run_bass_kernel_spmd(nc2, [dict(parts=parts, cnt=cntg)],
                                         core_ids=[0])
    val = np.float32(r2.results[0]["out"].reshape(-1)[0])
    return val, r1, aligned


def kernel(x, y, anchors, sampled):
    val, _, _ = _run(REAL, np.asarray(x), np.asarray(y), np.asarray(anchors),
                     np.asarray(sampled),
                     trace=os.environ.get("K_TRACE", "0") == "1")
    return np.asarray(val, dtype=np.float32)


# revision 18
# speedup vs baseline: 1.1046x; 1.0763x over previous
"""Node2Node supervised-contrastive loss on 8 Trainium2 NeuronCores.

Data-parallel over the sample table (classic row-gather pipeline):
  - Core c owns x rows [c*SL, (c+1)*SL); it normalizes its slice on-device
    into a bf16 table [rows, 128] in DRAM (128-wide rows: 256B gather
    descriptors, half the HBM traffic of the augmented 256-wide layout).
  - Every (anchor, sample) pair is routed host-side to the core owning the
    sampled row; pairs sit in columns of 128 (one sample per anchor of a
    128-anchor block).  Positive/validity masks are folded host-side into a
    two-plane bf16 mask preloaded to SBUF, so the kernel needs no labels:
    per chunk it gathers rows, multiplies with the block's anchor features,
    tree-reduces over D, exponentiates, and accumulates the three per-anchor
    sums (num/den/cnt) with masked multiplies.
  - Indices and masks are preloaded in full (no per-chunk control DMAs).
  - A second tiny launch combines the 8 cores' partials into the loss.
"""

import os
import sys

import numpy as np
import ml_dtypes

sys.path.insert(0, "/opt/trn_rl_repo")

import concourse.bass as bass
import concourse.bacc as bacc
import concourse.mybir as mybir
import concourse.tile as tile
from concourse import bass_utils

F32 = mybir.dt.float32
BF16 = mybir.dt.bfloat16
I16 = mybir.dt.int16
I32 = mybir.dt.int32
MUL = mybir.AluOpType.mult
ADD = mybir.AluOpType.add
SUB = mybir.AluOpType.subtract
EQ = mybir.AluOpType.is_equal
AFT = mybir.ActivationFunctionType


class CFG:
    def __init__(self, N=100000, D=128, A=4096, S=512, NC=8, TEMP=0.1, MT=36):
        self.N, self.D, self.A, self.S, self.NC, self.TEMP = N, D, A, S, NC, TEMP
        self.SL = N // NC                      # rows per slice
        self.NB = A // 128                     # anchor blocks (slots of 128)
        self.G = -(-self.SL // 128)            # slice col-groups of 128 rows
        self.SLP = self.G * 128                # padded slice rows
        self.MT = MT                           # max columns per gather call


REAL = CFG()


# --------------------------------------------------------------------------
# host-side index prep (pure numpy; integer bookkeeping only)
# --------------------------------------------------------------------------

def prep(cfg, x, y, anchors, sampled):
    N, A, S, NC, SL, NB = cfg.N, cfg.A, cfg.S, cfg.NC, cfg.SL, cfg.NB
    x = np.ascontiguousarray(np.asarray(x, dtype=np.float32))
    y64 = np.asarray(y, dtype=np.int64)
    anchors = np.asarray(anchors, dtype=np.int64)
    sampled = np.asarray(sampled, dtype=np.int64)

    posm = (y64[sampled] == y64[anchors][:, None])   # [A, S]

    core_of = sampled // SL                    # [A, S]
    cnt = np.zeros((A, NC), dtype=np.int64)
    for c in range(NC):
        cnt[:, c] = (core_of == c).sum(1)

    perms, ranks = [], []
    Ms = np.zeros((NC, NB), dtype=np.int64)
    for c in range(NC):
        p = np.argsort(cnt[:, c], kind="stable")
        r = np.empty(A, dtype=np.int64)
        r[p] = np.arange(A)
        perms.append(p)
        ranks.append(r)
        Ms[c] = cnt[p, c].reshape(NB, 128).max(1)
    M = Ms.max(0)                              # uniform per-block columns
    Cj = np.concatenate([[0], np.cumsum(M)])   # block column offsets
    MTOT = int(Cj[-1])

    cores = []
    for c in range(NC):
        perm, rank = perms[c], ranks[c]
        a_list, s_list = np.nonzero(core_of == c)       # sorted by anchor
        local = (sampled[a_list, s_list] - c * SL).astype(np.int64)
        order = np.lexsort((local, a_list))             # HBM locality
        a_list, s_list, local = a_list[order], s_list[order], local[order]
        n = cnt[:, c]
        start = np.concatenate([[0], np.cumsum(n)])
        k = np.arange(len(a_list)) - start[a_list]      # within-anchor pos
        r = rank[a_list]
        j, p = r // 128, r % 128
        col = Cj[j] + k
        idxmat = np.zeros((128, MTOT), dtype=np.int16)
        vmask = np.zeros((128, 2, MTOT), dtype=ml_dtypes.bfloat16)
        idxmat[p, col] = local.astype(np.int16)
        vmask[p, 0, col] = 1.0                          # valid
        vmask[p, 1, col] = posm[a_list, s_list].astype(np.float32)
        # flat gather list, column-major: position t = col*128 + p
        flat = idxmat.T.reshape(-1)                     # [MTOT*128]
        L = flat.size // 16
        wrapped = np.zeros((128, L), dtype=np.int16)
        w16 = flat.reshape(L, 16).T
        for g in range(8):
            wrapped[g * 16:(g + 1) * 16, :] = w16

        # anchor-side host data (slot order)
        aperm = anchors[perm]                           # [A] node ids
        xa = x[aperm].reshape(NB, 128, cfg.D).transpose(1, 0, 2).copy()

        # slice input (padded, bf16)
        xs = np.ones((cfg.SLP, cfg.D), dtype=ml_dtypes.bfloat16)
        xs[:SL] = x[c * SL:(c + 1) * SL].astype(ml_dtypes.bfloat16)

        cores.append(dict(xs=xs, xa=xa, sidx=wrapped, vmask=vmask))
    return cores, perms, M.astype(int).tolist()


# --------------------------------------------------------------------------
# kernel 1: per-core partial sums
# --------------------------------------------------------------------------

def build_k1(cfg, M, repeat=1):
    NB, D, G, SLP, MT = cfg.NB, cfg.D, cfg.G, cfg.SLP, cfg.MT
    MTOT = sum(M)
    nc = bacc.Bacc("TRN2", target_bir_lowering=False, debug=False,
                   num_devices=cfg.NC, num_swdge_queues=4)
    xs = nc.dram_tensor("xs", [SLP, D], BF16, kind="ExternalInput").ap()
    xa = nc.dram_tensor("xa", [128, NB, D], F32, kind="ExternalInput").ap()
    sidx = nc.dram_tensor("sidx", [128, MTOT * 8], I16, kind="ExternalInput").ap()
    vmask = nc.dram_tensor("vmask", [128, 2, MTOT], BF16,
                           kind="ExternalInput").ap()
    acc_out = nc.dram_tensor("acc", [128, NB, 3], F32, kind="ExternalOutput").ap()

    with tile.TileContext(nc) as tc:
        with tc.tile_pool(name="dram", bufs=1, space="DRAM") as dpool, \
             tc.tile_pool(name="pe", bufs=1) as pe:
            table = dpool.tile([SLP, D], BF16)
            sidxT = pe.tile([128, MTOT * 8], I16)
            nc.sync.dma_start(sidxT[:], sidx[:])
            vmT = pe.tile([128, 2, MTOT], BF16)
            nc.sync.dma_start(vmT[:], vmask[:])

            # ---- phase A: build normalized bf16 slice table ----
            with tc.tile_pool(name="pa", bufs=2) as pa:
                half = (G + 3) // 4
                xsr = xs.rearrange("(g p) d -> p g d", p=128)
                tbr = table[:].rearrange("(g p) e -> p g e", p=128)
                for h in range(4):
                    g0 = h * half
                    g1 = min(G, g0 + half)
                    gw = g1 - g0
                    if gw <= 0:
                        continue
                    xt = pa.tile([128, half, D], BF16, tag="xt")
                    nc.sync.dma_start(xt[:, :gw, :], xsr[:, g0:g1, :])
                    sq = pa.tile([128, half, D], F32, tag="sq")
                    nc.vector.tensor_tensor(out=sq[:, :gw, :], in0=xt[:, :gw, :],
                                            in1=xt[:, :gw, :], op=MUL)
                    ss = pa.tile([128, half], F32, tag="ss")
                    nc.vector.reduce_sum(out=ss[:, :gw], in_=sq[:, :gw, :],
                                         axis=mybir.AxisListType.X)
                    nc.scalar.activation(ss[:, :gw], ss[:, :gw], AFT.Sqrt)
                    inv = pa.tile([128, half], F32, tag="inv")
                    nc.vector.reciprocal(inv[:, :gw], ss[:, :gw])
                    tb = pa.tile([128, half, D], BF16, tag="tb")
                    nc.vector.tensor_tensor(
                        out=tb[:, :gw, :], in0=xt[:, :gw, :],
                        in1=inv[:, :gw].unsqueeze(2).to_broadcast([128, gw, D]),
                        op=MUL)
                    nc.sync.dma_start(tbr[:, g0:g1, :], tb[:, :gw, :])

            # ---- phase B: anchor features (slot layout) ----
            with tc.tile_pool(name="pb", bufs=1) as pb, \
                 tc.tile_pool(name="res", bufs=1) as res:
                xat = pb.tile([128, NB, D], F32)
                nc.sync.dma_start(xat[:], xa[:])
                sqa = pb.tile([128, NB, D], F32)
                nc.vector.tensor_tensor(out=sqa[:], in0=xat[:], in1=xat[:], op=MUL)
                ssa = pb.tile([128, NB], F32)
                nc.vector.reduce_sum(out=ssa[:], in_=sqa[:],
                                     axis=mybir.AxisListType.X)
                nc.scalar.activation(ssa[:], ssa[:], AFT.Sqrt)
                inva = pb.tile([128, NB], F32)
                nc.vector.reciprocal(inva[:], ssa[:])
                af = res.tile([128, NB, D], BF16)
                nc.vector.tensor_tensor(
                    out=af[:], in0=xat[:],
                    in1=inva[:].unsqueeze(2).to_broadcast([128, NB, D]), op=MUL)
                acc = res.tile([128, NB, 3], F32)
                nc.vector.memset(acc[:], 0.0)

                # ---- phase C: main pair loop ----
                with tc.tile_pool(name="pcb", bufs=1) as pcb, \
                     tc.tile_pool(name="pc", bufs=4) as pc:
                  for _rep in range(repeat):
                    Cj = 0
                    gq = 0
                    for j in range(NB):
                        mj = M[j]
                        c0 = 0
                        while c0 < mj:
                            mt = min(MT, mj - c0)
                            col = Cj + c0            # global column offset
                            st = pcb.tile([128, MT, D], BF16, tag=f"st{gq % 6}")
                            nc.gpsimd.dma_gather(
                                st[:, :mt, :], table[:],
                                sidxT[:, col * 8:(col + mt) * 8],
                                mt * 128, mt * 128, D, single_packet=False,
                                queue_num=gq % 4)
                            gq += 1
                            pr = pcb.tile([128, MT, D], BF16, tag=f"pr{gq % 3}")
                            nc.vector.tensor_tensor(
                                out=pr[:, :mt, :], in0=st[:, :mt, :],
                                in1=af[:, j:j + 1, :].to_broadcast([128, mt, D]),
                                op=MUL)
                            w = D // 2
                            while w >= 8:
                                nc.vector.tensor_tensor(
                                    out=pr[:, :mt, 0:w],
                                    in0=pr[:, :mt, 0:w],
                                    in1=pr[:, :mt, w:2 * w], op=ADD)
                                w //= 2
                            sd = pc.tile([128, MT], F32, tag="sd")
                            nc.vector.reduce_sum(out=sd[:, :mt],
                                                 in_=pr[:, :mt, 0:8],
                                                 axis=mybir.AxisListType.X)
                            e = pc.tile([128, MT], F32, tag="e")
                            nc.scalar.activation(e[:, :mt], sd[:, :mt],
                                                 AFT.Exp, scale=1.0 / cfg.TEMP)
                            rr = pc.tile([128, 3, MT], F32, tag="rr")
                            nc.vector.tensor_copy(out=rr[:, 2, :mt],
                                                  in_=vmT[:, 1, col:col + mt])
                            nc.vector.tensor_tensor(
                                out=rr[:, 1, :mt], in0=e[:, :mt],
                                in1=vmT[:, 0, col:col + mt], op=MUL)
                            nc.vector.tensor_tensor(
                                out=rr[:, 0, :mt], in0=e[:, :mt],
                                in1=vmT[:, 1, col:col + mt], op=MUL)
                            tmp = pc.tile([128, 3], F32, tag="tmp")
                            nc.vector.reduce_sum(out=tmp[:], in_=rr[:, :, :mt],
                                                 axis=mybir.AxisListType.X)
                            nc.vector.tensor_tensor(
                                out=acc[:, j, :], in0=acc[:, j, :],
                                in1=tmp[:], op=ADD)
                            c0 += mt
                        Cj += mj
                nc.sync.dma_start(acc_out[:], acc[:])
    nc.compile()
    return nc


# --------------------------------------------------------------------------
# kernel 2: combine partials, per-anchor loss, total
# --------------------------------------------------------------------------

def build_k2(cfg):
    NB, NC = cfg.NB, cfg.NC
    nc = bacc.Bacc("TRN2", target_bir_lowering=False, debug=False, num_devices=1)
    parts = nc.dram_tensor("parts", [128, NC, NB, 3], F32,
                           kind="ExternalInput").ap()
    out = nc.dram_tensor("out", [1, 1], F32, kind="ExternalOutput").ap()
    with tile.TileContext(nc) as tc:
        with tc.tile_pool(name="p", bufs=1) as p, \
             tc.tile_pool(name="ps", bufs=1, space="PSUM") as psp:
            t = p.tile([128, NC, NB, 3], F32)
            nc.sync.dma_start(t[:], parts[:])
            s3 = p.tile([128, NB, 3], F32)
            tt = t[:].transpose([0, 2, 3, 1])
            nc.vector.reduce_sum(out=s3[:], in_=tt, axis=mybir.AxisListType.X)
            n_ = s3[:, :, 0]
            d_ = s3[:, :, 1]
            c_ = s3[:, :, 2]
            cz = p.tile([128, NB], F32)
            nc.vector.tensor_scalar(out=cz[:], in0=c_, scalar1=0.0, scalar2=None,
                                    op0=EQ)
            n1 = p.tile([128, NB], F32)
            nc.vector.tensor_tensor(out=n1[:], in0=n_, in1=cz[:], op=ADD)
            c1 = p.tile([128, NB], F32)
            nc.vector.tensor_scalar_max(out=c1[:], in0=c_, scalar1=1.0)
            lnn = p.tile([128, NB], F32)
            nc.scalar.activation(lnn[:], n1[:], AFT.Ln)
            lnd = p.tile([128, NB], F32)
            nc.scalar.activation(lnd[:], d_, AFT.Ln)
            df = p.tile([128, NB], F32)
            nc.vector.tensor_tensor(out=df[:], in0=lnd[:], in1=lnn[:], op=SUB)
            rc = p.tile([128, NB], F32)
            nc.vector.reciprocal(rc[:], c1[:])
            pa = p.tile([128, NB], F32)
            nc.vector.tensor_tensor(out=pa[:], in0=df[:], in1=rc[:], op=MUL)
            m = p.tile([128, NB], F32)
            nc.scalar.activation(m[:], cz[:], AFT.Copy, scale=-1.0, bias=1.0)
            pa2 = p.tile([128, NB], F32)
            nc.vector.tensor_tensor(out=pa2[:], in0=pa[:], in1=m[:], op=MUL)
            rs = p.tile([128, 1], F32)
            nc.vector.reduce_sum(out=rs[:], in_=pa2[:], axis=mybir.AxisListType.X)
            ones = p.tile([128, 1], F32)
            nc.vector.memset(ones[:], 1.0)
            acc = psp.tile([1, 1], F32)
            nc.tensor.matmul(out=acc[:], lhsT=rs[:], rhs=ones[:], start=True,
                             stop=True)
            res = p.tile([1, 1], F32)
            nc.vector.tensor_copy(out=res[:], in_=acc[:])
            nc.sync.dma_start(out[:], res[:])
    nc.compile()
    return nc


# --------------------------------------------------------------------------
# entry point
# --------------------------------------------------------------------------

def _run(cfg, x, y, anchors, sampled, trace=False):
    cores, perms, M = prep(cfg, x, y, anchors, sampled)
    nc1 = build_k1(cfg, M)
    in_maps = [dict(xs=c["xs"], xa=c["xa"], sidx=c["sidx"], vmask=c["vmask"])
               for c in cores]
    kw = dict(trace=True, trace_cores=list(range(cfg.NC)), stitch_traces=False) \
        if trace else {}
    r1 = bass_utils.run_bass_kernel_spmd(nc1, in_maps,
                                         core_ids=list(range(cfg.NC)), **kw)
    aligned = np.zeros((cfg.NC, cfg.A, 3), dtype=np.float32)
    for c in range(cfg.NC):
        acc = r1.results[c]["acc"]                       # [128, NB, 3]
        acc_t = acc.transpose(1, 0, 2).reshape(cfg.A, 3)  # slot-rank order
        aligned[c, perms[c]] = acc_t
    parts = aligned.reshape(cfg.NC, cfg.NB, 128, 3).transpose(2, 0, 1, 3).copy()
    nc2 = build_k2(cfg)
    r2 = bass_utils.run_bass_kernel_spmd(nc2, [dict(parts=parts)], core_ids=[0])
    val = np.float32(r2.results[0]["out"].reshape(-1)[0])
    return val, r1, aligned


def kernel(x, y, anchors, sampled):
    val, _, _ = _run(REAL, np.asarray(x), np.asarray(y), np.asarray(anchors),
                     np.asarray(sampled),
                     trace=os.environ.get("K_TRACE", "0") == "1")
    return np.asarray(val, dtype=np.float32)


# revision 20
# speedup vs baseline: 1.9160x; 1.7345x over previous
"""Node2Node supervised-contrastive loss on 8 Trainium2 NeuronCores.

Strategy (data-parallel over the sample table):
  - The x table is split into 8 row-slices of N/8; core c owns slice c and
    normalizes it on-device into a bf16 "augmented" table [rows, 256] =
    [xn (128) | y (1) | zeros].
  - Every (anchor, sample) pair is routed (host-side index bookkeeping only)
    to the core owning the sampled row. Each core dma_gathers its pairs'
    rows, multiplies with the (device-normalized) anchor features, reduces
    over D with a binary tree on the vector engine, exponentiates, masks,
    and accumulates per-anchor partial numerator/denominator/count sums.
  - Pairs are laid out in "columns" of 128 (one per partition); anchors are
    grouped into 32 blocks of 128 slots so a column holds one sample of each
    of the block's anchors; per-anchor sums then become free-dim reductions.
  - A second tiny launch sums the 8 cores' per-anchor partials and computes
    -log(num/den)/cnt and the final scalar reduction on-device.
"""

import os
import sys

import numpy as np
import ml_dtypes

sys.path.insert(0, "/opt/trn_rl_repo")

import concourse.bass as bass
import concourse.bacc as bacc
import concourse.mybir as mybir
import concourse.tile as tile
from concourse import bass_utils

F32 = mybir.dt.float32
BF16 = mybir.dt.bfloat16
I16 = mybir.dt.int16
I32 = mybir.dt.int32
MUL = mybir.AluOpType.mult
ADD = mybir.AluOpType.add
SUB = mybir.AluOpType.subtract
EQ = mybir.AluOpType.is_equal
AFT = mybir.ActivationFunctionType


class CFG:
    def __init__(self, N=100000, D=128, A=4096, S=512, NC=8, TEMP=0.1, MT=36):
        self.N, self.D, self.A, self.S, self.NC, self.TEMP = N, D, A, S, NC, TEMP
        self.SL = N // NC                      # rows per slice
        self.NB = A // 128                     # anchor blocks (slots of 128)
        self.G = -(-self.SL // 128)            # slice col-groups of 128 rows
        self.SLP = self.G * 128                # padded slice rows
        self.MT = MT                           # max columns per gather call


REAL = CFG()


# --------------------------------------------------------------------------
# host-side index prep (pure numpy; integer bookkeeping only)
# --------------------------------------------------------------------------

def prep(cfg, x, y, anchors, sampled):
    N, A, S, NC, SL, NB = cfg.N, cfg.A, cfg.S, cfg.NC, cfg.SL, cfg.NB
    x = np.ascontiguousarray(np.asarray(x, dtype=np.float32))
    y64 = np.asarray(y, dtype=np.int64)
    anchors = np.asarray(anchors, dtype=np.int64)
    sampled = np.asarray(sampled, dtype=np.int64)

    core_of = sampled // SL                    # [A, S]
    # per (anchor, core) counts
    cnt = np.zeros((A, NC), dtype=np.int64)
    for c in range(NC):
        cnt[:, c] = (core_of == c).sum(1)

    # per-core anchor->slot permutation (sorted by count) and uniform block sizes
    perms, ranks = [], []
    Ms = np.zeros((NC, NB), dtype=np.int64)
    for c in range(NC):
        p = np.argsort(cnt[:, c], kind="stable")
        r = np.empty(A, dtype=np.int64)
        r[p] = np.arange(A)
        perms.append(p)
        ranks.append(r)
        Ms[c] = cnt[p, c].reshape(NB, 128).max(1)
    M = Ms.max(0)                              # uniform per-block columns
    Cj = np.concatenate([[0], np.cumsum(M)])   # block column offsets
    MTOT = int(Cj[-1])

    cores = []
    for c in range(NC):
        perm, rank = perms[c], ranks[c]
        a_list, s_list = np.nonzero(core_of == c)       # sorted by anchor
        local = (sampled[a_list, s_list] - c * SL).astype(np.int64)
        n = cnt[:, c]
        start = np.concatenate([[0], np.cumsum(n)])
        k = np.arange(len(a_list)) - start[a_list]      # within-anchor position
        r = rank[a_list]
        j, p = r // 128, r % 128
        col = Cj[j] + k
        idxmat = np.zeros((128, MTOT), dtype=np.int16)
        valid = np.zeros((128, MTOT), dtype=ml_dtypes.bfloat16)
        idxmat[p, col] = local.astype(np.int16)
        valid[p, col] = 1.0
        # flat gather list, column-major: position t = col*128 + p
        flat = idxmat.T.reshape(-1)                     # [MTOT*128]
        L = flat.size // 16
        wrapped = np.zeros((128, L), dtype=np.int16)
        w16 = flat.reshape(L, 16).T
        for g in range(8):
            wrapped[g * 16:(g + 1) * 16, :] = w16

        # anchor-side host data (slot order)
        aperm = anchors[perm]                           # [A] node ids, slot order
        xa = x[aperm].reshape(NB, 128, cfg.D).transpose(1, 0, 2).copy()  # [128,NB,D]
        ya = y64[aperm].astype(np.float32).reshape(NB, 128).T.copy()  # [128, NB]

        # slice inputs (padded)
        xs = np.ones((cfg.SLP, cfg.D), dtype=ml_dtypes.bfloat16)
        xs[:SL] = x[c * SL:(c + 1) * SL].astype(ml_dtypes.bfloat16)
        ysl = np.zeros((cfg.SLP, 2), dtype=np.int32)
        ysl[:SL] = y64[c * SL:(c + 1) * SL, None].view(np.int32).reshape(SL, 2)

        cores.append(dict(
            xs=xs, ys=ysl, xa=xa, ya=ya, sidx=wrapped, vmask=valid,
        ))
    return cores, perms, M.astype(int).tolist()


# --------------------------------------------------------------------------
# kernel 1: per-core partial sums
# --------------------------------------------------------------------------

def build_k1(cfg, M, repeat=1):
    NB, D, G, SLP, MT = cfg.NB, cfg.D, cfg.G, cfg.SLP, cfg.MT
    MTOT = sum(M)
    nc = bacc.Bacc("TRN2", target_bir_lowering=False, debug=False,
                   num_devices=cfg.NC, num_swdge_queues=4)
    xs = nc.dram_tensor("xs", [SLP, D], BF16, kind="ExternalInput").ap()
    ys = nc.dram_tensor("ys", [SLP, 2], I32, kind="ExternalInput").ap()
    xa = nc.dram_tensor("xa", [128, NB, D], F32, kind="ExternalInput").ap()
    ya_in = nc.dram_tensor("ya", [128, NB], F32, kind="ExternalInput").ap()
    sidx = nc.dram_tensor("sidx", [128, MTOT * 8], I16, kind="ExternalInput").ap()
    vmask = nc.dram_tensor("vmask", [128, MTOT], BF16, kind="ExternalInput").ap()
    acc_out = nc.dram_tensor("acc", [128, NB, 3], F32, kind="ExternalOutput").ap()

    with tile.TileContext(nc) as tc:
        with tc.tile_pool(name="dram", bufs=1, space="DRAM") as dpool, \
             tc.tile_pool(name="pe", bufs=1) as pe:
            table = dpool.tile([SLP, 256], BF16)

            # ---- phase A: build normalized augmented slice table ----
            with tc.tile_pool(name="pa", bufs=2) as pa:
                half = (G + 3) // 4
                xsr = xs.rearrange("(g p) d -> p g d", p=128)
                ysr = ys.rearrange("(g p) k -> p g k", p=128)
                tbr = table[:].rearrange("(g p) e -> p g e", p=128)
                for h in range(4):
                    g0 = h * half
                    g1 = min(G, g0 + half)
                    gw = g1 - g0
                    if gw <= 0:
                        continue
                    xt = pa.tile([128, half, D], BF16, tag="xt")
                    nc.sync.dma_start(xt[:, :gw, :], xsr[:, g0:g1, :])
                    sq = pa.tile([128, half, D], F32, tag="sq")
                    nc.vector.tensor_tensor(out=sq[:, :gw, :], in0=xt[:, :gw, :],
                                            in1=xt[:, :gw, :], op=MUL)
                    ss = pa.tile([128, half], F32, tag="ss")
                    nc.vector.reduce_sum(out=ss[:, :gw], in_=sq[:, :gw, :],
                                         axis=mybir.AxisListType.X)
                    nc.scalar.activation(ss[:, :gw], ss[:, :gw], AFT.Sqrt)
                    inv = pa.tile([128, half], F32, tag="inv")
                    nc.vector.reciprocal(inv[:, :gw], ss[:, :gw])
                    tb = pa.tile([128, half, 256], BF16, tag="tb")
                    nc.vector.tensor_tensor(
                        out=tb[:, :gw, 0:D], in0=xt[:, :gw, :],
                        in1=inv[:, :gw].unsqueeze(2).to_broadcast([128, gw, D]),
                        op=MUL)
                    yt = pa.tile([128, half, 2], I32, tag="yt")
                    nc.sync.dma_start(yt[:, :gw, :], ysr[:, g0:g1, :])
                    nc.vector.tensor_copy(out=tb[:, :gw, D:D + 1],
                                          in_=yt[:, :gw, 0:1])
                    nc.sync.dma_start(tbr[:, g0:g1, :], tb[:, :gw, :])

            # ---- phase B: anchor features + labels (slot layout) ----
            with tc.tile_pool(name="pb", bufs=1) as pb, \
                 tc.tile_pool(name="res", bufs=1) as res:
                xat = pb.tile([128, NB, D], F32)
                nc.sync.dma_start(xat[:], xa[:])
                sqa = pb.tile([128, NB, D], F32)
                nc.vector.tensor_tensor(out=sqa[:], in0=xat[:], in1=xat[:], op=MUL)
                ssa = pb.tile([128, NB], F32)
                nc.vector.reduce_sum(out=ssa[:], in_=sqa[:],
                                     axis=mybir.AxisListType.X)
                nc.scalar.activation(ssa[:], ssa[:], AFT.Sqrt)
                inva = pb.tile([128, NB], F32)
                nc.vector.reciprocal(inva[:], ssa[:])
                af = res.tile([128, NB, D], BF16)
                nc.vector.tensor_tensor(
                    out=af[:], in0=xat[:],
                    in1=inva[:].unsqueeze(2).to_broadcast([128, NB, D]), op=MUL)

                ya = res.tile([128, NB], F32)
                nc.sync.dma_start(ya[:], ya_in[:])
                acc = res.tile([128, NB, 3], F32)
                nc.vector.memset(acc[:], 0.0)

                # ---- phase C: main pair loop ----
                with tc.tile_pool(name="pcb", bufs=1) as pcb, \
                     tc.tile_pool(name="pc", bufs=4) as pc:
                  for _rep in range(repeat):
                    Cj = 0
                    gq = 0
                    for j in range(NB):
                        mj = M[j]
                        c0 = 0
                        while c0 < mj:
                            mt = min(MT, mj - c0)
                            col = Cj + c0            # global column offset
                            it = pc.tile([128, MT * 8], I16, tag=f"it{gq % 6}")
                            nc.sync.dma_start(
                                it[:, :mt * 8],
                                sidx[:, col * 8:(col + mt) * 8])
                            st = pcb.tile([128, MT, 256], BF16, tag=f"st{gq % 6}")
                            nc.gpsimd.dma_gather(
                                st[:, :mt, :], table[:], it[:, :mt * 8],
                                mt * 128, mt * 128, 256, single_packet=False,
                                queue_num=gq % 4)
                            gq += 1
                            ysd = pc.tile([128, MT], BF16, tag="ysd")
                            nc.scalar.activation(ysd[:, :mt], st[:, :mt, D],
                                                 AFT.Copy)
                            pr = st[:, :mt, D:2 * D]
                            nc.vector.tensor_tensor(
                                out=pr, in0=st[:, :mt, 0:D],
                                in1=af[:, j:j + 1, :].to_broadcast([128, mt, D]),
                                op=MUL)
                            w = D // 2
                            while w >= 8:
                                nc.vector.tensor_tensor(
                                    out=st[:, :mt, D:D + w],
                                    in0=st[:, :mt, D:D + w],
                                    in1=st[:, :mt, D + w:D + 2 * w], op=ADD)
                                w //= 2
                            # final 8-wide fold into a separate tile frees st
                            # (the gather buffer) before the rest of the chain
                            sd = pc.tile([128, MT], F32, tag="sd")
                            nc.vector.reduce_sum(out=sd[:, :mt],
                                                 in_=st[:, :mt, D:D + 8],
                                                 axis=mybir.AxisListType.X)
                            e = pc.tile([128, MT], F32, tag="e")
                            nc.scalar.activation(e[:, :mt], sd[:, :mt],
                                                 AFT.Exp, scale=1.0 / cfg.TEMP)
                            pm = pc.tile([128, MT], BF16, tag="pm")
                            nc.vector.tensor_tensor(
                                out=pm[:, :mt], in0=ysd[:, :mt],
                                in1=ya[:, j:j + 1].to_broadcast([128, mt]),
                                op=EQ)
                            vm = pc.tile([128, MT], BF16, tag="vm")
                            nc.sync.dma_start(vm[:, :mt], vmask[:, col:col + mt])
                            rr = pc.tile([128, 3, MT], F32, tag="rr")
                            nc.vector.tensor_tensor(out=rr[:, 2, :mt],
                                                    in0=pm[:, :mt],
                                                    in1=vm[:, :mt], op=MUL)
                            nc.vector.tensor_tensor(out=rr[:, 1, :mt],
                                                    in0=e[:, :mt],
                                                    in1=vm[:, :mt], op=MUL)
                            nc.vector.tensor_tensor(out=rr[:, 0, :mt],
                                                    in0=rr[:, 1, :mt],
                                                    in1=rr[:, 2, :mt], op=MUL)
                            tmp = pc.tile([128, 3], F32, tag="tmp")
                            nc.vector.reduce_sum(out=tmp[:], in_=rr[:, :, :mt],
                                                 axis=mybir.AxisListType.X)
                            nc.vector.tensor_tensor(
                                out=acc[:, j, :], in0=acc[:, j, :],
                                in1=tmp[:], op=ADD)
                            c0 += mt
                        Cj += mj
                nc.sync.dma_start(acc_out[:], acc[:])
    nc.compile()
    return nc


# --------------------------------------------------------------------------
# kernel 2: combine partials, per-anchor loss, total
# --------------------------------------------------------------------------

def build_k2(cfg):
    NB, NC = cfg.NB, cfg.NC
    nc = bacc.Bacc("TRN2", target_bir_lowering=False, debug=False, num_devices=1)
    parts = nc.dram_tensor("parts", [128, NC, NB, 3], F32,
                           kind="ExternalInput").ap()
    out = nc.dram_tensor("out", [1, 1], F32, kind="ExternalOutput").ap()
    with tile.TileContext(nc) as tc:
        with tc.tile_pool(name="p", bufs=1) as p, \
             tc.tile_pool(name="ps", bufs=1, space="PSUM") as psp:
            t = p.tile([128, NC, NB, 3], F32)
            nc.sync.dma_start(t[:], parts[:])
            s3 = p.tile([128, NB, 3], F32)
            # sum over the core axis (stride NB*3 innermost)
            tt = t[:].transpose([0, 2, 3, 1])
            nc.vector.reduce_sum(out=s3[:], in_=tt, axis=mybir.AxisListType.X)
            n_ = s3[:, :, 0]
            d_ = s3[:, :, 1]
            c_ = s3[:, :, 2]
            cz = p.tile([128, NB], F32)
            nc.vector.tensor_scalar(out=cz[:], in0=c_, scalar1=0.0, scalar2=None,
                                    op0=EQ)
            n1 = p.tile([128, NB], F32)
            nc.vector.tensor_tensor(out=n1[:], in0=n_, in1=cz[:], op=ADD)
            c1 = p.tile([128, NB], F32)
            nc.vector.tensor_scalar_max(out=c1[:], in0=c_, scalar1=1.0)
            lnn = p.tile([128, NB], F32)
            nc.scalar.activation(lnn[:], n1[:], AFT.Ln)
            lnd = p.tile([128, NB], F32)
            nc.scalar.activation(lnd[:], d_, AFT.Ln)
            df = p.tile([128, NB], F32)
            nc.vector.tensor_tensor(out=df[:], in0=lnd[:], in1=lnn[:], op=SUB)
            rc = p.tile([128, NB], F32)
            nc.vector.reciprocal(rc[:], c1[:])
            pa = p.tile([128, NB], F32)
            nc.vector.tensor_tensor(out=pa[:], in0=df[:], in1=rc[:], op=MUL)
            m = p.tile([128, NB], F32)
            nc.scalar.activation(m[:], cz[:], AFT.Copy, scale=-1.0, bias=1.0)
            pa2 = p.tile([128, NB], F32)
            nc.vector.tensor_tensor(out=pa2[:], in0=pa[:], in1=m[:], op=MUL)
            rs = p.tile([128, 1], F32)
            nc.vector.reduce_sum(out=rs[:], in_=pa2[:], axis=mybir.AxisListType.X)
            ones = p.tile([128, 1], F32)
            nc.vector.memset(ones[:], 1.0)
            acc = psp.tile([1, 1], F32)
            nc.tensor.matmul(out=acc[:], lhsT=rs[:], rhs=ones[:], start=True,
                             stop=True)
            res = p.tile([1, 1], F32)
            nc.vector.tensor_copy(out=res[:], in_=acc[:])
            nc.sync.dma_start(out[:], res[:])
    nc.compile()
    return nc


# --------------------------------------------------------------------------
# entry point
# --------------------------------------------------------------------------

def _run(cfg, x, y, anchors, sampled, trace=False):
    cores, perms, M = prep(cfg, x, y, anchors, sampled)
    nc1 = build_k1(cfg, M)
    in_maps = [dict(xs=c["xs"], ys=c["ys"], xa=c["xa"], ya=c["ya"],
                    sidx=c["sidx"], vmask=c["vmask"])
               for c in cores]
    kw = dict(trace=True, trace_cores=list(range(cfg.NC)), stitch_traces=False) \
        if trace else {}
    r1 = bass_utils.run_bass_kernel_spmd(nc1, in_maps,
                                         core_ids=list(range(cfg.NC)), **kw)
    # realign slot-order partials to anchor order (host: pure indexing)
    aligned = np.zeros((cfg.NC, cfg.A, 3), dtype=np.float32)
    for c in range(cfg.NC):
        acc = r1.results[c]["acc"]                       # [128, NB, 3]
        acc_t = acc.transpose(1, 0, 2).reshape(cfg.A, 3)  # slot-rank order
        aligned[c, perms[c]] = acc_t
    parts = aligned.reshape(cfg.NC, cfg.NB, 128, 3).transpose(2, 0, 1, 3).copy()
    nc2 = build_k2(cfg)
    r2 = bass_utils.run_bass_kernel_spmd(nc2, [dict(parts=parts)], core_ids=[0])
    val = np.float32(r2.results[0]["out"].reshape(-1)[0])
    return val, r1, aligned


def kernel(x, y, anchors, sampled):
    val, _, _ = _run(REAL, np.asarray(x), np.asarray(y), np.asarray(anchors),
                     np.asarray(sampled),
                     trace=os.environ.get("K_TRACE", "0") == "1")
    return np.asarray(val, dtype=np.float32)

